# revision 21
# baseline (speedup 1.0000x reference)
"""Trainium2 Bass kernel for nn_NeuralField_18605798326294.

NeRF-style MLP over N=524288 query points, data-parallel over 8 NeuronCores.

Per-core layout is feature-major ([features, points]) so every layer is a
single PE matmul with the weight matrix stationary:
  out[f_out, n] = W[f_in, f_out].T @ act[f_in, n]
The 224-wide concat-skip contraction is split into two accumulating matmuls
(h part K=128 + enc part K=96) into the same PSUM bank group.

Frequency encoding (rows in the reference feature order j = c*32 + k):
  t   = x_c * 2^(l-1) + (0.25 if cos else 0)    exact in fp32
  u   = t - round(t)  in [-0.5, 0.5]            round via +/- 1.5*2^23 magic
  enc = Sin(2*pi * u)                           ACT, scale folds the 2*pi

Matmuls run in bf16 (weights, enc, h quantized; PSUM accumulates fp32):
the 32-bit moving operand streams at half rate on HW, bf16 at full rate.

Points are processed as 64 half-chunks of HC=1024 (2 PSUM banks of f32 per
activation tile). Four half-chunk streams advance layer-by-layer in
lockstep: while the PE runs one stream's matmuls for layer l, the other
three streams' relu+bias run on ACT (even streams) / DVE (odd streams), so
each relu has a ~2.6us window against its ~1.3us latency and the PE never
waits. The t/kk/u range reduction runs on GpSimd (Pool) to keep DVE under
the PE's per-step budget; the next group's encodes are hoisted into layer 3
of the current group to hide the group transition.
"""
import sys
sys.path.insert(0, "/opt/trn_rl_repo")
import numpy as np

N = 524288
NCORES = 8
NPC = N // NCORES          # 65536 points per core
NT = 512                   # points per matmul (one PSUM bank of f32)
HC = 2048                  # points per half-chunk (4 PSUM banks)
MMF = 512                  # matmul free size (points per matmul)
GT = HC // MMF             # matmul tiles per half-chunk
NSTREAM = 2                # half-chunk streams in lockstep (fills 8 banks)
NGROUP = NPC // (NSTREAM * HC)   # 16 groups per rep
L_FREQ = 16
DIM_ENC = 96
DIM_HID = 128
NUM_HID = 7
DIM_OUT = 4

MAGIC = float(np.float32(1.5 * 2 ** 23))
TWO_PI = float(np.float32(2 * np.pi))

_CACHE = {}


def _legalize_single_wait(nc, mybir):
    """This walrus build accepts only one sync wait per instruction; hoist
    extras into standalone EventSemaphore instructions just before the
    offender (same engine => sequencer order preserves semantics)."""
    for f in nc.m.functions:
        for b in f.blocks:
            out = []
            for inst in b.instructions:
                si = inst.sync_info
                if si is not None and len(si.on_wait) > 1:
                    waits = list(si.on_wait)
                    for k, w in enumerate(waits[:-1]):
                        out.append(mybir.InstEventSemaphore(
                            name=f"{inst.name}_w{k}", engine=inst.engine,
                            sync_info=mybir.SyncInfo(on_wait=[w], on_update=[]),
                        ))
                    inst.sync_info = mybir.SyncInfo(
                        on_wait=[waits[-1]], on_update=list(si.on_update))
                out.append(inst)
            b.instructions = out


def _build(reps=1):
    import concourse.bass as bass
    import concourse.mybir as mybir
    from concourse.tile import TileContext

    F32 = mybir.dt.float32
    BF16 = mybir.dt.bfloat16
    Sin = mybir.ActivationFunctionType.Sin
    Relu = mybir.ActivationFunctionType.Relu
    Copy = mybir.ActivationFunctionType.Copy
    AL = mybir.AluOpType

    nc = bass.Bass()
    xt = nc.declare_dram_parameter("xt", [3, NPC], F32, isOutput=False)
    w0 = nc.declare_dram_parameter("w0", [DIM_ENC, DIM_HID], BF16, isOutput=False)
    whh = nc.declare_dram_parameter("whh", [DIM_HID, NUM_HID * DIM_HID], BF16, isOutput=False)
    whe = nc.declare_dram_parameter("whe", [DIM_ENC, NUM_HID * DIM_HID], BF16, isOutput=False)
    wlh = nc.declare_dram_parameter("wlh", [DIM_HID, DIM_OUT], BF16, isOutput=False)
    wle = nc.declare_dram_parameter("wle", [DIM_ENC, DIM_OUT], BF16, isOutput=False)
    cols = nc.declare_dram_parameter("cols", [DIM_ENC, 3], F32, isOutput=False)
    b0 = nc.declare_dram_parameter("b0", [DIM_HID, 1], F32, isOutput=False)
    bh = nc.declare_dram_parameter("bh", [DIM_HID, NUM_HID], F32, isOutput=False)
    bl = nc.declare_dram_parameter("bl", [DIM_OUT, 1], F32, isOutput=False)
    y = nc.declare_dram_parameter("y", [DIM_OUT, NPC], F32, isOutput=True)

    with TileContext(nc) as tc:
        with tc.tile_pool(name="consts", bufs=1) as consts, \
             tc.tile_pool(name="sb", bufs=2) as sb, \
             tc.tile_pool(name="encp", bufs=2) as encp, \
             tc.tile_pool(name="hbuf", bufs=4) as hbuf, \
             tc.tile_pool(name="ps", bufs=2, space="PSUM") as ps:
            w0_sb = consts.tile([DIM_ENC, DIM_HID], BF16)
            nc.sync.dma_start(out=w0_sb[:], in_=w0[:])
            whh_sb = consts.tile([DIM_HID, NUM_HID * DIM_HID], BF16)
            nc.sync.dma_start(out=whh_sb[:], in_=whh[:])
            whe_sb = consts.tile([DIM_ENC, NUM_HID * DIM_HID], BF16)
            nc.sync.dma_start(out=whe_sb[:], in_=whe[:])
            wlh_sb = consts.tile([DIM_HID, DIM_OUT], BF16)
            nc.sync.dma_start(out=wlh_sb[:], in_=wlh[:])
            wle_sb = consts.tile([DIM_ENC, DIM_OUT], BF16)
            nc.sync.dma_start(out=wle_sb[:], in_=wle[:])
            col_sb = consts.tile([DIM_ENC, 3], F32)
            nc.sync.dma_start(out=col_sb[:], in_=cols[:])
            b0_sb = consts.tile([DIM_HID, 1], F32)
            nc.sync.dma_start(out=b0_sb[:], in_=b0[:])
            bh_sb = consts.tile([DIM_HID, NUM_HID], F32)
            nc.sync.dma_start(out=bh_sb[:], in_=bh[:])
            bl_sb = consts.tile([DIM_OUT, 1], F32)
            nc.sync.dma_start(out=bl_sb[:], in_=bl[:])

            def encode(pair, fast=False):
                """DMA + frequency-encode one pair (both streams' points) as
                single wide [96, 2*HC] ops; returns the enc tile.
                fast=True runs the chain on DVE (startup latency); otherwise
                on GpSimd to keep DVE free for relus."""
                eng = nc.vector if fast else nc.gpsimd
                W = NSTREAM * HC
                base = xt[:, pair * W:(pair + 1) * W]
                bc = bass.AP(tensor=base.tensor, offset=base.offset,
                             ap=[base.ap[0], [0, 32], base.ap[1]])
                xrep = sb.tile([DIM_ENC, W], F32, tag="xrep")
                nc.sync.dma_start(out=xrep[:], in_=bc)
                t = sb.tile([DIM_ENC, W], F32, tag="t")
                eng.tensor_scalar(t[:], xrep[:], col_sb[:, 0:1],
                                  col_sb[:, 1:2], AL.mult, AL.add)
                kk = sb.tile([DIM_ENC, W], F32, tag="kk")
                eng.tensor_scalar(kk[:], t[:], MAGIC, MAGIC,
                                  AL.add, AL.subtract)
                eng.tensor_tensor(t[:], t[:], kk[:], AL.subtract)  # u in-place
                enc = encp.tile([DIM_ENC, W], BF16, tag="enc")
                nc.scalar.activation(enc[:], t[:], Sin, scale=TWO_PI)
                return enc

            def mm_layer(l, h, enc, s):
                """PE matmuls for stream s, one layer; returns PSUM tile.
                enc is the pair-wide tile; stream s uses columns [s*HC:...]"""
                p = ps.tile([DIM_HID, HC], F32, tag="p")
                e0 = s * HC
                if l == 0:
                    for g in range(GT):
                        sl = slice(g * MMF, (g + 1) * MMF)
                        esl = slice(e0 + g * MMF, e0 + (g + 1) * MMF)
                        nc.tensor.matmul(p[:, sl], w0_sb[:], enc[:, esl],
                                         start=True, stop=True)
                elif l <= NUM_HID:
                    wsl = slice((l - 1) * DIM_HID, l * DIM_HID)
                    for g in range(GT):
                        sl = slice(g * MMF, (g + 1) * MMF)
                        nc.tensor.matmul(p[:, sl], whh_sb[:, wsl], h[:, sl],
                                         start=True, stop=False)
                    for g in range(GT):
                        sl = slice(g * MMF, (g + 1) * MMF)
                        esl = slice(e0 + g * MMF, e0 + (g + 1) * MMF)
                        nc.tensor.matmul(p[:, sl], whe_sb[:, wsl], enc[:, esl],
                                         start=False, stop=True)
                else:
                    for g in range(GT):
                        sl = slice(g * MMF, (g + 1) * MMF)
                        nc.tensor.matmul(p[:4, sl], wlh_sb[:], h[:, sl],
                                         start=True, stop=False)
                    for g in range(GT):
                        sl = slice(g * MMF, (g + 1) * MMF)
                        esl = slice(e0 + g * MMF, e0 + (g + 1) * MMF)
                        nc.tensor.matmul(p[:4, sl], wle_sb[:], enc[:, esl],
                                         start=False, stop=True)
                return p

            def act_layer(l, p, s, yt):
                """bias + relu (or final bias copy) for one stream/layer."""
                if l <= NUM_HID:
                    bias = b0_sb[:] if l == 0 else bh_sb[:, l - 1:l]
                    h = hbuf.tile([DIM_HID, HC], BF16, tag="h")
                    if s % 2 == 0:
                        nc.scalar.activation(h[:], p[:], Relu, bias=bias)
                    else:
                        nc.vector.tensor_scalar(h[:], p[:], bias, 0.0,
                                                AL.add, AL.max)
                    return h
                nc.vector.tensor_scalar_add(yt[:, s * HC:(s + 1) * HC],
                                            p[:4, :], bl_sb[:])
                return None

            HOIST = 3  # layer after which the next group's encodes are issued

            def group_body(grp, enc):
                """One group = NSTREAM half-chunk streams in lockstep."""
                hs = [None] * NSTREAM
                enc_next = None
                W = NSTREAM * HC
                yt = sb.tile([DIM_OUT, W], F32, tag="yt")
                for l in range(NUM_HID + 2):
                    if l == HOIST and grp + 1 < NGROUP:
                        enc_next = encode(grp + 1)
                    ps_l = [mm_layer(l, hs[s], enc, s)
                            for s in range(NSTREAM)]
                    for s in range(NSTREAM):
                        hs[s] = act_layer(l, ps_l[s], s, yt)
                nc.sync.dma_start(out=y[:, grp * W:(grp + 1) * W], in_=yt[:])
                return enc_next

            def all_groups():
                enc = encode(0, fast=True)
                for grp in range(NGROUP):
                    enc = group_body(grp, enc)

            if reps == 1:
                all_groups()
            else:
                # reps as a hardware loop: NEFF size stays constant across
                # reps so the marginal bench measures execution, not NEFF load
                with tc.For_i(0, reps, 1):
                    all_groups()

    _legalize_single_wait(nc, mybir)
    return nc


def _prep_shared(W0, b0, Wh, bh, Wl, bl):
    scale = np.zeros((DIM_ENC,), np.float32)
    shift = np.zeros((DIM_ENC,), np.float32)
    for c in range(3):
        for k in range(32):
            j = c * 32 + k
            l = k if k < L_FREQ else k - L_FREQ
            scale[j] = np.float32(2.0 ** (l - 1))
            shift[j] = np.float32(0.0 if k < L_FREQ else 0.25)
    pi_col = np.full((DIM_ENC,), np.float32(np.pi), np.float32)
    cols = np.stack([scale, shift, pi_col], axis=1)  # [96, 3]

    whh = np.ascontiguousarray(
        np.concatenate([Wh[i][:DIM_HID] for i in range(NUM_HID)], axis=1))
    whe = np.ascontiguousarray(
        np.concatenate([Wh[i][DIM_HID:] for i in range(NUM_HID)], axis=1))
    import ml_dtypes
    bf16 = ml_dtypes.bfloat16
    return {
        "w0": np.ascontiguousarray(W0).astype(bf16),
        "whh": whh.astype(bf16),
        "whe": whe.astype(bf16),
        "wlh": np.ascontiguousarray(Wl[:DIM_HID]).astype(bf16),
        "wle": np.ascontiguousarray(Wl[DIM_HID:]).astype(bf16),
        "cols": cols.astype(np.float32),
        "b0": np.ascontiguousarray(b0.reshape(DIM_HID, 1), np.float32),
        "bh": np.ascontiguousarray(bh.T, np.float32),           # [128, 7]
        "bl": np.ascontiguousarray(bl.reshape(DIM_OUT, 1), np.float32),
    }


def _get_nc(reps=1):
    key = ("nc", reps)
    if key not in _CACHE:
        _CACHE[key] = _build(reps=reps)
    return _CACHE[key]


def _get_runner(reps=1):
    """Compile the Bass module to a cached jitted shard_map callable.

    run_bass_kernel_spmd builds a fresh jit closure per call (full re-trace +
    XLA compile every time); caching the callable keeps repeat kernel() calls
    on the fast path."""
    key = ("runner", reps)
    if key in _CACHE:
        return _CACHE[key]

    import jax
    import concourse.mybir as mybir
    from concourse import bass2jax
    from jax.sharding import Mesh, PartitionSpec
    from jax.experimental.shard_map import shard_map

    bass2jax.install_neuronx_cc_hook()
    nc = _get_nc(reps=reps)

    part_name = nc.partition_id_tensor.name if nc.partition_id_tensor else None
    in_names, out_names, out_avals = [], [], []
    for alloc in nc.m.functions[0].allocations:
        if not isinstance(alloc, mybir.MemoryLocationSet):
            continue
        name = alloc.memorylocations[0].name
        if alloc.kind == "ExternalInput":
            if name != part_name:
                in_names.append(name)
        elif alloc.kind == "ExternalOutput":
            out_names.append(name)
            out_avals.append(jax.core.ShapedArray(
                tuple(alloc.tensor_shape), mybir.dt.np(alloc.dtype)))
    n_params = len(in_names)
    all_names = in_names + out_names
    if part_name is not None:
        all_names = all_names + [part_name]

    def _body(*args):
        operands = list(args)
        if part_name is not None:
            operands.append(bass2jax.partition_id_tensor())
        outs = bass2jax._bass_exec_p.bind(
            *operands,
            out_avals=tuple(out_avals),
            in_names=tuple(all_names),
            out_names=tuple(out_names),
            lowering_input_output_aliases=(),
            sim_require_finite=True,
            sim_require_nnan=True,
            nc=nc,
        )
        return tuple(outs)

    donate = tuple(range(n_params, n_params + len(out_names)))
    devices = jax.devices()[:NCORES]
    mesh = Mesh(np.asarray(devices), ("core",))
    n_in = n_params + len(out_names)
    sharded = jax.jit(
        shard_map(_body, mesh=mesh,
                  in_specs=(PartitionSpec("core"),) * n_in,
                  out_specs=(PartitionSpec("core"),) * len(out_names),
                  check_rep=False),
        donate_argnums=donate, keep_unused=True)
    runner = (sharded, in_names, out_names, out_avals)
    _CACHE[key] = runner
    return runner


def kernel(query_points, W0, b0, Wh, bh, Wl, bl, _reps=1):
    sharded, in_names, out_names, out_avals = _get_runner(reps=_reps)

    shared = _prep_shared(np.asarray(W0), np.asarray(b0), np.asarray(Wh),
                          np.asarray(bh), np.asarray(Wl), np.asarray(bl))
    xt = np.ascontiguousarray(np.asarray(query_points, np.float32).T)  # [3, N]

    # global input layout: per-core shards concatenated on axis 0
    concat_in = []
    for name in in_names:
        if name == "xt":
            concat_in.append(np.ascontiguousarray(
                xt.reshape(3, NCORES, NPC).transpose(1, 0, 2)
            ).reshape(NCORES * 3, NPC))
        else:
            v = shared[name]
            concat_in.append(np.concatenate([v] * NCORES, axis=0))
    concat_zeros = [
        np.zeros((NCORES * a.shape[0],) + tuple(a.shape[1:]), a.dtype)
        for a in out_avals
    ]
    out_arrs = sharded(*concat_in, *concat_zeros)
    yg = np.asarray(out_arrs[out_names.index("y")])          # [8*4, NPC]
    full = yg.reshape(NCORES, DIM_OUT, NPC)
    out = np.ascontiguousarray(
        full.transpose(0, 2, 1).reshape(N, DIM_OUT), np.float32)  # [N, 4]
    return out


# revision 22
# speedup vs baseline: 1.4844x; 1.4844x over previous
"""Trainium2 Bass kernel for nn_NeuralField_18605798326294.

NeRF-style MLP over N=524288 query points, data-parallel over 8 NeuronCores.

Per-core layout is feature-major ([features, points]) so every layer is a
single PE matmul with the weight matrix stationary:
  out[f_out, n] = W[f_in, f_out].T @ act[f_in, n]
The 224-wide concat-skip contraction is split into two accumulating matmuls
(h part K=128 + enc part K=96) into the same PSUM bank group.

Frequency encoding (rows in the reference feature order j = c*32 + k):
  t   = x_c * 2^(l-1) + (0.25 if cos else 0)    exact in fp32
  u   = t - round(t)  in [-0.5, 0.5]            round via +/- 1.5*2^23 magic
  enc = Sin(2*pi * u)                           ACT, scale folds the 2*pi

Matmuls run in bf16 (weights, enc, h quantized; PSUM accumulates fp32):
the 32-bit moving operand streams at half rate on HW, bf16 at full rate.

Points are processed as 64 half-chunks of HC=1024 (2 PSUM banks of f32 per
activation tile). Four half-chunk streams advance layer-by-layer in
lockstep: while the PE runs one stream's matmuls for layer l, the other
three streams' relu+bias run on ACT (even streams) / DVE (odd streams), so
each relu has a ~2.6us window against its ~1.3us latency and the PE never
waits. The t/kk/u range reduction runs on GpSimd (Pool) to keep DVE under
the PE's per-step budget; the next group's encodes are hoisted into layer 3
of the current group to hide the group transition.
"""
import sys
sys.path.insert(0, "/opt/trn_rl_repo")
import numpy as np

N = 524288
NCORES = 8
NPC = N // NCORES          # 65536 points per core
NT = 512                   # points per matmul (one PSUM bank of f32)
HC = 2048                  # points per half-chunk (4 PSUM banks)
MMF = 512                  # matmul free size (points per matmul)
GT = HC // MMF             # matmul tiles per half-chunk
NSTREAM = 2                # half-chunk streams in lockstep (fills 8 banks)
NGROUP = NPC // (NSTREAM * HC)   # 16 groups per rep
L_FREQ = 16
DIM_ENC = 96
DIM_HID = 128
NUM_HID = 7
DIM_OUT = 4

MAGIC = float(np.float32(1.5 * 2 ** 23))
TWO_PI = float(np.float32(2 * np.pi))

_CACHE = {}


def _legalize_single_wait(nc, mybir):
    """This walrus build accepts only one sync wait per instruction; hoist
    extras into standalone EventSemaphore instructions just before the
    offender (same engine => sequencer order preserves semantics)."""
    for f in nc.m.functions:
        for b in f.blocks:
            out = []
            for inst in b.instructions:
                si = inst.sync_info
                if si is not None and len(si.on_wait) > 1:
                    waits = list(si.on_wait)
                    for k, w in enumerate(waits[:-1]):
                        out.append(mybir.InstEventSemaphore(
                            name=f"{inst.name}_w{k}", engine=inst.engine,
                            sync_info=mybir.SyncInfo(on_wait=[w], on_update=[]),
                        ))
                    inst.sync_info = mybir.SyncInfo(
                        on_wait=[waits[-1]], on_update=list(si.on_update))
                out.append(inst)
            b.instructions = out


def _build(reps=1):
    import concourse.bass as bass
    import concourse.mybir as mybir
    from concourse.tile import TileContext

    F32 = mybir.dt.float32
    BF16 = mybir.dt.bfloat16
    Sin = mybir.ActivationFunctionType.Sin
    Relu = mybir.ActivationFunctionType.Relu
    Copy = mybir.ActivationFunctionType.Copy
    AL = mybir.AluOpType

    nc = bass.Bass()
    xt = nc.declare_dram_parameter("xt", [3, NPC], F32, isOutput=False)
    w0 = nc.declare_dram_parameter("w0", [DIM_ENC, DIM_HID], BF16, isOutput=False)
    whh = nc.declare_dram_parameter("whh", [DIM_HID, NUM_HID * DIM_HID], BF16, isOutput=False)
    whe = nc.declare_dram_parameter("whe", [DIM_ENC, NUM_HID * DIM_HID], BF16, isOutput=False)
    wlh = nc.declare_dram_parameter("wlh", [DIM_HID, DIM_OUT], BF16, isOutput=False)
    wle = nc.declare_dram_parameter("wle", [DIM_ENC, DIM_OUT], BF16, isOutput=False)
    cols = nc.declare_dram_parameter("cols", [DIM_ENC, 3], F32, isOutput=False)
    b0 = nc.declare_dram_parameter("b0", [DIM_HID, 1], F32, isOutput=False)
    bh = nc.declare_dram_parameter("bh", [DIM_HID, NUM_HID], F32, isOutput=False)
    bl = nc.declare_dram_parameter("bl", [DIM_OUT, 1], F32, isOutput=False)
    y = nc.declare_dram_parameter("y", [DIM_OUT, NPC], F32, isOutput=True)

    with TileContext(nc) as tc:
        with tc.tile_pool(name="consts", bufs=1) as consts, \
             tc.tile_pool(name="sb", bufs=2) as sb, \
             tc.tile_pool(name="encp", bufs=2) as encp, \
             tc.tile_pool(name="hbuf", bufs=4) as hbuf, \
             tc.tile_pool(name="ps", bufs=2, space="PSUM") as ps:
            w0_sb = consts.tile([DIM_ENC, DIM_HID], BF16)
            nc.sync.dma_start(out=w0_sb[:], in_=w0[:])
            whh_sb = consts.tile([DIM_HID, NUM_HID * DIM_HID], BF16)
            nc.sync.dma_start(out=whh_sb[:], in_=whh[:])
            whe_sb = consts.tile([DIM_ENC, NUM_HID * DIM_HID], BF16)
            nc.sync.dma_start(out=whe_sb[:], in_=whe[:])
            wlh_sb = consts.tile([DIM_HID, DIM_OUT], BF16)
            nc.sync.dma_start(out=wlh_sb[:], in_=wlh[:])
            wle_sb = consts.tile([DIM_ENC, DIM_OUT], BF16)
            nc.sync.dma_start(out=wle_sb[:], in_=wle[:])
            col_sb = consts.tile([DIM_ENC, 3], F32)
            nc.sync.dma_start(out=col_sb[:], in_=cols[:])
            b0_sb = consts.tile([DIM_HID, 1], F32)
            nc.sync.dma_start(out=b0_sb[:], in_=b0[:])
            bh_sb = consts.tile([DIM_HID, NUM_HID], F32)
            nc.sync.dma_start(out=bh_sb[:], in_=bh[:])
            bl_sb = consts.tile([DIM_OUT, 1], F32)
            nc.sync.dma_start(out=bl_sb[:], in_=bl[:])

            def encode(pair, fast=False):
                """DMA + frequency-encode one pair (both streams' points) as
                single wide [96, 2*HC] ops; returns the enc tile.
                fast=True runs the chain on DVE (startup latency); otherwise
                on GpSimd to keep DVE free for relus."""
                ueng = nc.vector if fast else nc.gpsimd
                W = NSTREAM * HC
                base = xt[:, pair * W:(pair + 1) * W]
                bc = bass.AP(tensor=base.tensor, offset=base.offset,
                             ap=[base.ap[0], [0, 32], base.ap[1]])
                xrep = sb.tile([DIM_ENC, W], F32, tag="xrep")
                nc.sync.dma_start(out=xrep[:], in_=bc)
                # t/kk on DVE (2x-mode tensor_scalar, cheap); u on GpSimd
                # (HW Pool ops are slow, but a single hoisted one hides)
                t = sb.tile([DIM_ENC, W], F32, tag="t")
                nc.vector.tensor_scalar(t[:], xrep[:], col_sb[:, 0:1],
                                        col_sb[:, 1:2], AL.mult, AL.add)
                kk = sb.tile([DIM_ENC, W], F32, tag="kk")
                nc.vector.tensor_scalar(kk[:], t[:], MAGIC, MAGIC,
                                        AL.add, AL.subtract)
                ueng.tensor_tensor(t[:], t[:], kk[:], AL.subtract)  # u in-place
                enc = encp.tile([DIM_ENC, W], BF16, tag="enc")
                nc.scalar.activation(enc[:], t[:], Sin, scale=TWO_PI)
                return enc

            def mm_layer(l, h, enc, s):
                """PE matmuls for stream s, one layer; returns PSUM tile.
                enc is the pair-wide tile; stream s uses columns [s*HC:...]"""
                p = ps.tile([DIM_HID, HC], F32, tag="p")
                e0 = s * HC
                if l == 0:
                    for g in range(GT):
                        sl = slice(g * MMF, (g + 1) * MMF)
                        esl = slice(e0 + g * MMF, e0 + (g + 1) * MMF)
                        nc.tensor.matmul(p[:, sl], w0_sb[:], enc[:, esl],
                                         start=True, stop=True)
                elif l <= NUM_HID:
                    wsl = slice((l - 1) * DIM_HID, l * DIM_HID)
                    for g in range(GT):
                        sl = slice(g * MMF, (g + 1) * MMF)
                        nc.tensor.matmul(p[:, sl], whh_sb[:, wsl], h[:, sl],
                                         start=True, stop=False)
                    for g in range(GT):
                        sl = slice(g * MMF, (g + 1) * MMF)
                        esl = slice(e0 + g * MMF, e0 + (g + 1) * MMF)
                        nc.tensor.matmul(p[:, sl], whe_sb[:, wsl], enc[:, esl],
                                         start=False, stop=True)
                else:
                    for g in range(GT):
                        sl = slice(g * MMF, (g + 1) * MMF)
                        nc.tensor.matmul(p[:4, sl], wlh_sb[:], h[:, sl],
                                         start=True, stop=False)
                    for g in range(GT):
                        sl = slice(g * MMF, (g + 1) * MMF)
                        esl = slice(e0 + g * MMF, e0 + (g + 1) * MMF)
                        nc.tensor.matmul(p[:4, sl], wle_sb[:], enc[:, esl],
                                         start=False, stop=True)
                return p

            def act_layer(l, p, s, yt):
                """bias + relu (or final bias copy) for one stream/layer."""
                if l <= NUM_HID:
                    bias = b0_sb[:] if l == 0 else bh_sb[:, l - 1:l]
                    h = hbuf.tile([DIM_HID, HC], BF16, tag="h")
                    if s % 2 == 0:
                        nc.scalar.activation(h[:], p[:], Relu, bias=bias)
                    else:
                        nc.vector.tensor_scalar(h[:], p[:], bias, 0.0,
                                                AL.add, AL.max)
                    return h
                nc.vector.tensor_scalar_add(yt[:, s * HC:(s + 1) * HC],
                                            p[:4, :], bl_sb[:])
                return None

            HOIST = 3  # layer after which the next group's encodes are issued

            def group_body(grp, enc):
                """One group = NSTREAM half-chunk streams in lockstep."""
                hs = [None] * NSTREAM
                enc_next = None
                W = NSTREAM * HC
                yt = sb.tile([DIM_OUT, W], F32, tag="yt")
                for l in range(NUM_HID + 2):
                    if l == HOIST and grp + 1 < NGROUP:
                        enc_next = encode(grp + 1)
                    ps_l = [mm_layer(l, hs[s], enc, s)
                            for s in range(NSTREAM)]
                    for s in range(NSTREAM):
                        hs[s] = act_layer(l, ps_l[s], s, yt)
                nc.sync.dma_start(out=y[:, grp * W:(grp + 1) * W], in_=yt[:])
                return enc_next

            def all_groups():
                enc = encode(0, fast=True)
                for grp in range(NGROUP):
                    enc = group_body(grp, enc)

            if reps == 1:
                all_groups()
            else:
                # reps as a hardware loop: NEFF size stays constant across
                # reps so the marginal bench measures execution, not NEFF load
                with tc.For_i(0, reps, 1):
                    all_groups()

    _legalize_single_wait(nc, mybir)
    return nc


def _prep_shared(W0, b0, Wh, bh, Wl, bl):
    scale = np.zeros((DIM_ENC,), np.float32)
    shift = np.zeros((DIM_ENC,), np.float32)
    for c in range(3):
        for k in range(32):
            j = c * 32 + k
            l = k if k < L_FREQ else k - L_FREQ
            scale[j] = np.float32(2.0 ** (l - 1))
            shift[j] = np.float32(0.0 if k < L_FREQ else 0.25)
    pi_col = np.full((DIM_ENC,), np.float32(np.pi), np.float32)
    cols = np.stack([scale, shift, pi_col], axis=1)  # [96, 3]

    whh = np.ascontiguousarray(
        np.concatenate([Wh[i][:DIM_HID] for i in range(NUM_HID)], axis=1))
    whe = np.ascontiguousarray(
        np.concatenate([Wh[i][DIM_HID:] for i in range(NUM_HID)], axis=1))
    import ml_dtypes
    bf16 = ml_dtypes.bfloat16
    return {
        "w0": np.ascontiguousarray(W0).astype(bf16),
        "whh": whh.astype(bf16),
        "whe": whe.astype(bf16),
        "wlh": np.ascontiguousarray(Wl[:DIM_HID]).astype(bf16),
        "wle": np.ascontiguousarray(Wl[DIM_HID:]).astype(bf16),
        "cols": cols.astype(np.float32),
        "b0": np.ascontiguousarray(b0.reshape(DIM_HID, 1), np.float32),
        "bh": np.ascontiguousarray(bh.T, np.float32),           # [128, 7]
        "bl": np.ascontiguousarray(bl.reshape(DIM_OUT, 1), np.float32),
    }


def _get_nc(reps=1):
    key = ("nc", reps)
    if key not in _CACHE:
        _CACHE[key] = _build(reps=reps)
    return _CACHE[key]


def _get_runner(reps=1):
    """Compile the Bass module to a cached jitted shard_map callable.

    run_bass_kernel_spmd builds a fresh jit closure per call (full re-trace +
    XLA compile every time); caching the callable keeps repeat kernel() calls
    on the fast path."""
    key = ("runner", reps)
    if key in _CACHE:
        return _CACHE[key]

    import jax
    import concourse.mybir as mybir
    from concourse import bass2jax
    from jax.sharding import Mesh, PartitionSpec
    from jax.experimental.shard_map import shard_map

    bass2jax.install_neuronx_cc_hook()
    nc = _get_nc(reps=reps)

    part_name = nc.partition_id_tensor.name if nc.partition_id_tensor else None
    in_names, out_names, out_avals = [], [], []
    for alloc in nc.m.functions[0].allocations:
        if not isinstance(alloc, mybir.MemoryLocationSet):
            continue
        name = alloc.memorylocations[0].name
        if alloc.kind == "ExternalInput":
            if name != part_name:
                in_names.append(name)
        elif alloc.kind == "ExternalOutput":
            out_names.append(name)
            out_avals.append(jax.core.ShapedArray(
                tuple(alloc.tensor_shape), mybir.dt.np(alloc.dtype)))
    n_params = len(in_names)
    all_names = in_names + out_names
    if part_name is not None:
        all_names = all_names + [part_name]

    def _body(*args):
        operands = list(args)
        if part_name is not None:
            operands.append(bass2jax.partition_id_tensor())
        outs = bass2jax._bass_exec_p.bind(
            *operands,
            out_avals=tuple(out_avals),
            in_names=tuple(all_names),
            out_names=tuple(out_names),
            lowering_input_output_aliases=(),
            sim_require_finite=True,
            sim_require_nnan=True,
            nc=nc,
        )
        return tuple(outs)

    donate = tuple(range(n_params, n_params + len(out_names)))
    devices = jax.devices()[:NCORES]
    mesh = Mesh(np.asarray(devices), ("core",))
    n_in = n_params + len(out_names)
    sharded = jax.jit(
        shard_map(_body, mesh=mesh,
                  in_specs=(PartitionSpec("core"),) * n_in,
                  out_specs=(PartitionSpec("core"),) * len(out_names),
                  check_rep=False),
        donate_argnums=donate, keep_unused=True)
    runner = (sharded, in_names, out_names, out_avals)
    _CACHE[key] = runner
    return runner


def kernel(query_points, W0, b0, Wh, bh, Wl, bl, _reps=1):
    sharded, in_names, out_names, out_avals = _get_runner(reps=_reps)

    shared = _prep_shared(np.asarray(W0), np.asarray(b0), np.asarray(Wh),
                          np.asarray(bh), np.asarray(Wl), np.asarray(bl))
    xt = np.ascontiguousarray(np.asarray(query_points, np.float32).T)  # [3, N]

    # global input layout: per-core shards concatenated on axis 0
    concat_in = []
    for name in in_names:
        if name == "xt":
            concat_in.append(np.ascontiguousarray(
                xt.reshape(3, NCORES, NPC).transpose(1, 0, 2)
            ).reshape(NCORES * 3, NPC))
        else:
            v = shared[name]
            concat_in.append(np.concatenate([v] * NCORES, axis=0))
    concat_zeros = [
        np.zeros((NCORES * a.shape[0],) + tuple(a.shape[1:]), a.dtype)
        for a in out_avals
    ]
    out_arrs = sharded(*concat_in, *concat_zeros)
    yg = np.asarray(out_arrs[out_names.index("y")])          # [8*4, NPC]
    full = yg.reshape(NCORES, DIM_OUT, NPC)
    out = np.ascontiguousarray(
        full.transpose(0, 2, 1).reshape(N, DIM_OUT), np.float32)  # [N, 4]
    return out


# revision 23
# speedup vs baseline: 2.0531x; 1.3831x over previous
"""Trainium2 Bass kernel for nn_NeuralField_18605798326294.

NeRF-style MLP over N=524288 query points, data-parallel over 8 NeuronCores.

Per-core layout is feature-major ([features, points]) so every layer is a
single PE matmul with the weight matrix stationary:
  out[f_out, n] = W[f_in, f_out].T @ act[f_in, n]
The 224-wide concat-skip contraction is split into two accumulating matmuls
(h part K=128 + enc part K=96) into the same PSUM bank group.

Frequency encoding (rows in the reference feature order j = c*32 + k):
  t   = x_c * 2^(l-1) + (0.25 if cos else 0)    exact in fp32
  u   = t - round(t)  in [-0.5, 0.5]            round via +/- 1.5*2^23 magic
  enc = Sin(2*pi * u)                           ACT, scale folds the 2*pi

Matmuls run in bf16 (weights, enc, h quantized; PSUM accumulates fp32):
the 32-bit moving operand streams at half rate on HW, bf16 at full rate.

Points are processed as 64 half-chunks of HC=1024 (2 PSUM banks of f32 per
activation tile). Four half-chunk streams advance layer-by-layer in
lockstep: while the PE runs one stream's matmuls for layer l, the other
three streams' relu+bias run on ACT (even streams) / DVE (odd streams), so
each relu has a ~2.6us window against its ~1.3us latency and the PE never
waits. The t/kk/u range reduction runs on GpSimd (Pool) to keep DVE under
the PE's per-step budget; the next group's encodes are hoisted into layer 3
of the current group to hide the group transition.
"""
import sys
sys.path.insert(0, "/opt/trn_rl_repo")
import numpy as np

N = 524288
NCORES = 8
NPC = N // NCORES          # 65536 points per core
NT = 512                   # points per matmul (one PSUM bank of f32)
HC = 2048                  # points per half-chunk (4 PSUM banks)
MMF = 512                  # matmul free size (points per matmul)
GT = HC // MMF             # matmul tiles per half-chunk
NSTREAM = 2                # half-chunk streams in lockstep (fills 8 banks)
NGROUP = NPC // (NSTREAM * HC)   # 16 groups per rep
L_FREQ = 16
DIM_ENC = 96
DIM_HID = 128
NUM_HID = 7
DIM_OUT = 4

MAGIC = float(np.float32(1.5 * 2 ** 23))
TWO_PI = float(np.float32(2 * np.pi))

_CACHE = {}


def _legalize_single_wait(nc, mybir):
    """This walrus build accepts only one sync wait per instruction; hoist
    extras into standalone EventSemaphore instructions just before the
    offender (same engine => sequencer order preserves semantics)."""
    for f in nc.m.functions:
        for b in f.blocks:
            out = []
            for inst in b.instructions:
                si = inst.sync_info
                if si is not None and len(si.on_wait) > 1:
                    waits = list(si.on_wait)
                    for k, w in enumerate(waits[:-1]):
                        out.append(mybir.InstEventSemaphore(
                            name=f"{inst.name}_w{k}", engine=inst.engine,
                            sync_info=mybir.SyncInfo(on_wait=[w], on_update=[]),
                        ))
                    inst.sync_info = mybir.SyncInfo(
                        on_wait=[waits[-1]], on_update=list(si.on_update))
                out.append(inst)
            b.instructions = out


def _build(reps=1):
    import concourse.bass as bass
    import concourse.mybir as mybir
    from concourse.tile import TileContext

    F32 = mybir.dt.float32
    BF16 = mybir.dt.bfloat16
    Sin = mybir.ActivationFunctionType.Sin
    Relu = mybir.ActivationFunctionType.Relu
    Copy = mybir.ActivationFunctionType.Copy
    AL = mybir.AluOpType

    nc = bass.Bass()
    xt = nc.declare_dram_parameter("xt", [3, NPC], F32, isOutput=False)
    w0 = nc.declare_dram_parameter("w0", [DIM_ENC, DIM_HID], BF16, isOutput=False)
    whh = nc.declare_dram_parameter("whh", [DIM_HID, NUM_HID * DIM_HID], BF16, isOutput=False)
    whe = nc.declare_dram_parameter("whe", [DIM_ENC, NUM_HID * DIM_HID], BF16, isOutput=False)
    wlh = nc.declare_dram_parameter("wlh", [DIM_HID, DIM_OUT], BF16, isOutput=False)
    wle = nc.declare_dram_parameter("wle", [DIM_ENC, DIM_OUT], BF16, isOutput=False)
    cols = nc.declare_dram_parameter("cols", [DIM_ENC, 3], F32, isOutput=False)
    b0 = nc.declare_dram_parameter("b0", [DIM_HID, 1], F32, isOutput=False)
    bh = nc.declare_dram_parameter("bh", [DIM_HID, NUM_HID], F32, isOutput=False)
    bl = nc.declare_dram_parameter("bl", [DIM_OUT, 1], F32, isOutput=False)
    y = nc.declare_dram_parameter("y", [DIM_OUT, NPC], F32, isOutput=True)

    with TileContext(nc) as tc:
        with tc.tile_pool(name="consts", bufs=1) as consts, \
             tc.tile_pool(name="sb", bufs=2) as sb, \
             tc.tile_pool(name="encp", bufs=2) as encp, \
             tc.tile_pool(name="hbuf", bufs=4) as hbuf, \
             tc.tile_pool(name="ps", bufs=2, space="PSUM") as ps:
            w0_sb = consts.tile([DIM_ENC, DIM_HID], BF16)
            nc.sync.dma_start(out=w0_sb[:], in_=w0[:])
            whh_sb = consts.tile([DIM_HID, NUM_HID * DIM_HID], BF16)
            nc.sync.dma_start(out=whh_sb[:], in_=whh[:])
            whe_sb = consts.tile([DIM_ENC, NUM_HID * DIM_HID], BF16)
            nc.sync.dma_start(out=whe_sb[:], in_=whe[:])
            wlh_sb = consts.tile([DIM_HID, DIM_OUT], BF16)
            nc.sync.dma_start(out=wlh_sb[:], in_=wlh[:])
            wle_sb = consts.tile([DIM_ENC, DIM_OUT], BF16)
            nc.sync.dma_start(out=wle_sb[:], in_=wle[:])
            col_sb = consts.tile([DIM_ENC, 3], F32)
            nc.sync.dma_start(out=col_sb[:], in_=cols[:])
            b0_sb = consts.tile([DIM_HID, 1], F32)
            nc.sync.dma_start(out=b0_sb[:], in_=b0[:])
            bh_sb = consts.tile([DIM_HID, NUM_HID], F32)
            nc.sync.dma_start(out=bh_sb[:], in_=bh[:])
            bl_sb = consts.tile([DIM_OUT, 1], F32)
            nc.sync.dma_start(out=bl_sb[:], in_=bl[:])

            def encode(pair, fast=False):
                """DMA + frequency-encode one pair (both streams' points) as
                single wide [96, 2*HC] ops; returns the enc tile.
                fast=True runs the chain on DVE (startup latency); otherwise
                on GpSimd to keep DVE free for relus."""
                ueng = nc.vector if fast else nc.gpsimd
                W = NSTREAM * HC
                base = xt[:, pair * W:(pair + 1) * W]
                bc = bass.AP(tensor=base.tensor, offset=base.offset,
                             ap=[base.ap[0], [0, 32], base.ap[1]])
                xrep = sb.tile([DIM_ENC, W], F32, tag="xrep")
                nc.sync.dma_start(out=xrep[:], in_=bc)
                # t/kk on DVE (2x-mode tensor_scalar, cheap); u on GpSimd
                # (HW Pool ops are slow, but a single hoisted one hides)
                t = sb.tile([DIM_ENC, W], F32, tag="t")
                nc.vector.tensor_scalar(t[:], xrep[:], col_sb[:, 0:1],
                                        col_sb[:, 1:2], AL.mult, AL.add)
                kk = sb.tile([DIM_ENC, W], F32, tag="kk")
                nc.vector.tensor_scalar(kk[:], t[:], MAGIC, MAGIC,
                                        AL.add, AL.subtract)
                ueng.tensor_tensor(t[:], t[:], kk[:], AL.subtract)  # u in-place
                enc = encp.tile([DIM_ENC, W], BF16, tag="enc")
                nc.scalar.activation(enc[:], t[:], Sin, scale=TWO_PI)
                return enc

            def mm_layer(l, h, enc, s):
                """PE matmuls for stream s, one layer; returns PSUM tile.
                enc is the pair-wide tile; stream s uses columns [s*HC:...]"""
                p = ps.tile([DIM_HID, HC], F32, tag="p")
                e0 = s * HC
                if l == 0:
                    for g in range(GT):
                        sl = slice(g * MMF, (g + 1) * MMF)
                        esl = slice(e0 + g * MMF, e0 + (g + 1) * MMF)
                        nc.tensor.matmul(p[:, sl], w0_sb[:], enc[:, esl],
                                         start=True, stop=True)
                elif l <= NUM_HID:
                    wsl = slice((l - 1) * DIM_HID, l * DIM_HID)
                    for g in range(GT):
                        sl = slice(g * MMF, (g + 1) * MMF)
                        nc.tensor.matmul(p[:, sl], whh_sb[:, wsl], h[:, sl],
                                         start=True, stop=False)
                    for g in range(GT):
                        sl = slice(g * MMF, (g + 1) * MMF)
                        esl = slice(e0 + g * MMF, e0 + (g + 1) * MMF)
                        nc.tensor.matmul(p[:, sl], whe_sb[:, wsl], enc[:, esl],
                                         start=False, stop=True)
                else:
                    for g in range(GT):
                        sl = slice(g * MMF, (g + 1) * MMF)
                        nc.tensor.matmul(p[:4, sl], wlh_sb[:], h[:, sl],
                                         start=True, stop=False)
                    for g in range(GT):
                        sl = slice(g * MMF, (g + 1) * MMF)
                        esl = slice(e0 + g * MMF, e0 + (g + 1) * MMF)
                        nc.tensor.matmul(p[:4, sl], wle_sb[:], enc[:, esl],
                                         start=False, stop=True)
                return p

            def act_layer(l, p, s, yt):
                """bias + relu (or final bias copy) for one stream/layer.
                The relu is split across ACT and DVE so the two halves run in
                parallel, halving the latency the next matmuls wait on."""
                if l <= NUM_HID:
                    bias = b0_sb[:] if l == 0 else bh_sb[:, l - 1:l]
                    h = hbuf.tile([DIM_HID, HC], BF16, tag="h")
                    lo, hi = slice(0, HC // 2), slice(HC // 2, HC)
                    a_sl, v_sl = (lo, hi) if s % 2 == 0 else (hi, lo)
                    nc.scalar.activation(h[:, a_sl], p[:, a_sl], Relu,
                                         bias=bias)
                    nc.vector.tensor_scalar(h[:, v_sl], p[:, v_sl], bias,
                                            0.0, AL.add, AL.max)
                    return h
                nc.vector.tensor_scalar_add(yt[:, s * HC:(s + 1) * HC],
                                            p[:4, :], bl_sb[:])
                return None

            HOIST = 3  # layer after which the next group's encodes are issued

            def group_body(grp, enc):
                """One group = NSTREAM half-chunk streams in lockstep."""
                hs = [None] * NSTREAM
                enc_next = None
                W = NSTREAM * HC
                yt = sb.tile([DIM_OUT, W], F32, tag="yt")
                for l in range(NUM_HID + 2):
                    if l == HOIST and grp + 1 < NGROUP:
                        enc_next = encode(grp + 1)
                    ps_l = [mm_layer(l, hs[s], enc, s)
                            for s in range(NSTREAM)]
                    for s in range(NSTREAM):
                        hs[s] = act_layer(l, ps_l[s], s, yt)
                nc.sync.dma_start(out=y[:, grp * W:(grp + 1) * W], in_=yt[:])
                return enc_next

            def all_groups():
                enc = encode(0, fast=True)
                for grp in range(NGROUP):
                    enc = group_body(grp, enc)

            if reps == 1:
                all_groups()
            else:
                # reps as a hardware loop: NEFF size stays constant across
                # reps so the marginal bench measures execution, not NEFF load
                with tc.For_i(0, reps, 1):
                    all_groups()

    _legalize_single_wait(nc, mybir)
    return nc


def _prep_shared(W0, b0, Wh, bh, Wl, bl):
    scale = np.zeros((DIM_ENC,), np.float32)
    shift = np.zeros((DIM_ENC,), np.float32)
    for c in range(3):
        for k in range(32):
            j = c * 32 + k
            l = k if k < L_FREQ else k - L_FREQ
            scale[j] = np.float32(2.0 ** (l - 1))
            shift[j] = np.float32(0.0 if k < L_FREQ else 0.25)
    pi_col = np.full((DIM_ENC,), np.float32(np.pi), np.float32)
    cols = np.stack([scale, shift, pi_col], axis=1)  # [96, 3]

    whh = np.ascontiguousarray(
        np.concatenate([Wh[i][:DIM_HID] for i in range(NUM_HID)], axis=1))
    whe = np.ascontiguousarray(
        np.concatenate([Wh[i][DIM_HID:] for i in range(NUM_HID)], axis=1))
    import ml_dtypes
    bf16 = ml_dtypes.bfloat16
    return {
        "w0": np.ascontiguousarray(W0).astype(bf16),
        "whh": whh.astype(bf16),
        "whe": whe.astype(bf16),
        "wlh": np.ascontiguousarray(Wl[:DIM_HID]).astype(bf16),
        "wle": np.ascontiguousarray(Wl[DIM_HID:]).astype(bf16),
        "cols": cols.astype(np.float32),
        "b0": np.ascontiguousarray(b0.reshape(DIM_HID, 1), np.float32),
        "bh": np.ascontiguousarray(bh.T, np.float32),           # [128, 7]
        "bl": np.ascontiguousarray(bl.reshape(DIM_OUT, 1), np.float32),
    }


def _get_nc(reps=1):
    key = ("nc", reps)
    if key not in _CACHE:
        _CACHE[key] = _build(reps=reps)
    return _CACHE[key]


def _get_runner(reps=1):
    """Compile the Bass module to a cached jitted shard_map callable.

    run_bass_kernel_spmd builds a fresh jit closure per call (full re-trace +
    XLA compile every time); caching the callable keeps repeat kernel() calls
    on the fast path."""
    key = ("runner", reps)
    if key in _CACHE:
        return _CACHE[key]

    import jax
    import concourse.mybir as mybir
    from concourse import bass2jax
    from jax.sharding import Mesh, PartitionSpec
    from jax.experimental.shard_map import shard_map

    bass2jax.install_neuronx_cc_hook()
    nc = _get_nc(reps=reps)

    part_name = nc.partition_id_tensor.name if nc.partition_id_tensor else None
    in_names, out_names, out_avals = [], [], []
    for alloc in nc.m.functions[0].allocations:
        if not isinstance(alloc, mybir.MemoryLocationSet):
            continue
        name = alloc.memorylocations[0].name
        if alloc.kind == "ExternalInput":
            if name != part_name:
                in_names.append(name)
        elif alloc.kind == "ExternalOutput":
            out_names.append(name)
            out_avals.append(jax.core.ShapedArray(
                tuple(alloc.tensor_shape), mybir.dt.np(alloc.dtype)))
    n_params = len(in_names)
    all_names = in_names + out_names
    if part_name is not None:
        all_names = all_names + [part_name]

    def _body(*args):
        operands = list(args)
        if part_name is not None:
            operands.append(bass2jax.partition_id_tensor())
        outs = bass2jax._bass_exec_p.bind(
            *operands,
            out_avals=tuple(out_avals),
            in_names=tuple(all_names),
            out_names=tuple(out_names),
            lowering_input_output_aliases=(),
            sim_require_finite=True,
            sim_require_nnan=True,
            nc=nc,
        )
        return tuple(outs)

    donate = tuple(range(n_params, n_params + len(out_names)))
    devices = jax.devices()[:NCORES]
    mesh = Mesh(np.asarray(devices), ("core",))
    n_in = n_params + len(out_names)
    sharded = jax.jit(
        shard_map(_body, mesh=mesh,
                  in_specs=(PartitionSpec("core"),) * n_in,
                  out_specs=(PartitionSpec("core"),) * len(out_names),
                  check_rep=False),
        donate_argnums=donate, keep_unused=True)
    runner = (sharded, in_names, out_names, out_avals)
    _CACHE[key] = runner
    return runner


def kernel(query_points, W0, b0, Wh, bh, Wl, bl, _reps=1):
    sharded, in_names, out_names, out_avals = _get_runner(reps=_reps)

    shared = _prep_shared(np.asarray(W0), np.asarray(b0), np.asarray(Wh),
                          np.asarray(bh), np.asarray(Wl), np.asarray(bl))
    xt = np.ascontiguousarray(np.asarray(query_points, np.float32).T)  # [3, N]

    # global input layout: per-core shards concatenated on axis 0
    concat_in = []
    for name in in_names:
        if name == "xt":
            concat_in.append(np.ascontiguousarray(
                xt.reshape(3, NCORES, NPC).transpose(1, 0, 2)
            ).reshape(NCORES * 3, NPC))
        else:
            v = shared[name]
            concat_in.append(np.concatenate([v] * NCORES, axis=0))
    concat_zeros = [
        np.zeros((NCORES * a.shape[0],) + tuple(a.shape[1:]), a.dtype)
        for a in out_avals
    ]
    out_arrs = sharded(*concat_in, *concat_zeros)
    yg = np.asarray(out_arrs[out_names.index("y")])          # [8*4, NPC]
    full = yg.reshape(NCORES, DIM_OUT, NPC)
    out = np.ascontiguousarray(
        full.transpose(0, 2, 1).reshape(N, DIM_OUT), np.float32)  # [N, 4]
    return out
